# revision 14
# baseline (speedup 1.0000x reference)
"""Trainium2 Bass kernel for nn_ConditionalMoELayer (expert-parallel, sparse).

Two-phase design following the expert-parallel sharding hint:

Phase 1 (routing nets, data-parallel): the 8192 tokens are split across the
8 cores (1024 each). Each core runs the difficulty net and the gate on its
tokens in fp32r and ships the raw difficulty logit z and gate logits back.
The host finishes the (tiny, [8192 x 4]) discrete routing math in fp64:
k(z) via monotone softplus-inverted thresholds, top-k by rank, masked
softmax -> dense_w. Tokens whose z or logit-gap sits within 1e-3 of a
decision boundary (a handful) are recomputed exactly in fp64 so the
discrete decisions match the fp32 reference despite fp32r matmul noise.

Host dispatch (the "all-to-all"): tokens are grouped by assigned expert
(dense_w[t,e] > 0); each expert's token set is split over 2 cores
(4 experts x 2 = 8 cores), zero-padded to a common static capacity C.

Phase 2 (expert FFN, expert-parallel): each core holds ONE expert's weights
and computes y = relu(x @ W1 + b1) @ W2 for its gathered tokens in fp32r
(single-pass PE, full rate). All matmuls have 512-wide moving operands so
the 4-byte weight loads stay hidden; activations ride the gpsimd DMA queue
so they never wait behind weight loads. Host scales rows by dense_w and
scatter-adds into the output (token sets within one expert are disjoint).

Only ~40% of token-expert pairs are active, so phase 2 does ~2.5x fewer
FLOPs than the dense equivalent.
"""

import numpy as np

P = 128          # partitions
D = 1024         # d_model
H = 2048         # expert hidden
E = 4            # experts
H1 = 512         # difficulty-net hidden
T_FULL = 8192    # total tokens
N_CORES = 8
TC = T_FULL // N_CORES   # tokens per core in phase 1

KD = D // P      # 8   k-subtiles over D
KH = H // P      # 16  k-subtiles over H
MH1 = H1 // P    # 4   m-tiles of difficulty hidden
NT = TC // P     # 8   token tiles per core (phase 1)
ND = D // P      # 8   d-tiles (phase 2 output)
TCH = 512        # token chunk for matmul moving operand
NTCH = TC // TCH # 2

TH_LO = 0.5
TH_HI = 2.0
MIN_E = 1
RISK_MARGIN = 1e-3   # fp32r logit noise is ~3e-4 worst case; 3x cushion


def _ap_name(t):
    return t.tensor.name if hasattr(t, "tensor") else t.name


def build_routing_nc():
    """Phase-1 module: difficulty-net z and gate logits (transposed out)."""
    import concourse.mybir as mybir
    import concourse.tile as tile
    from concourse import bacc
    from contextlib import ExitStack

    f32 = mybir.dt.float32
    f32r = mybir.dt.float32r
    AF = mybir.ActivationFunctionType

    nc = bacc.Bacc(None, target_bir_lowering=False, debug=False)

    with tile.TileContext(nc) as tc:
        with ExitStack() as ctx:
            dram = ctx.enter_context(tc.tile_pool(name="dram", bufs=1, space="DRAM"))
            xt_d = dram.tile([P, KD, TC], f32r, kind="ExternalInput", name="xt")
            dp1t_d = dram.tile([P, KD, H1], f32r, kind="ExternalInput", name="dp1t")
            dp2t_d = dram.tile([P, MH1], f32r, kind="ExternalInput", name="dp2t")
            gwt_d = dram.tile([P, KD, E], f32r, kind="ExternalInput", name="gwt")
            dpb1_d = dram.tile([P, MH1], f32, kind="ExternalInput", name="dpb1")
            z_d = dram.tile([1, TC], f32, kind="ExternalOutput", name="zt")
            lt_d = dram.tile([E, TC], f32, kind="ExternalOutput", name="lt")

            const = ctx.enter_context(tc.tile_pool(name="const", bufs=1))
            xtp = ctx.enter_context(tc.tile_pool(name="xtp", bufs=1))
            h1p = ctx.enter_context(tc.tile_pool(name="h1p", bufs=2))
            outp = ctx.enter_context(tc.tile_pool(name="outp", bufs=1))
            psb = ctx.enter_context(tc.tile_pool(name="psb", bufs=4, space="PSUM"))
            pslp = ctx.enter_context(tc.tile_pool(name="pslp", bufs=2, space="PSUM"))
            psep = ctx.enter_context(tc.tile_pool(name="psep", bufs=2, space="PSUM"))

            xt_sb = xtp.tile([P, KD, TC], f32r, tag="xt")
            dp1t_sb = const.tile([P, KD, H1], f32r, tag="dp1t")
            dp2t_sb = const.tile([P, MH1], f32r, tag="dp2t")
            gwt_sb = const.tile([P, KD, E], f32r, tag="gwt")
            dpb1_sb = const.tile([P, MH1], f32, tag="dpb1")
            z_sb = outp.tile([1, TC], f32, tag="zsb")
            lt_sb = outp.tile([E, TC], f32, tag="ltsb")

            nc.sync.dma_start(gwt_sb[:], gwt_d[:])
            nc.sync.dma_start(dpb1_sb[:], dpb1_d[:])
            nc.sync.dma_start(dp2t_sb[:], dp2t_d[:])
            for mt in range(MH1):
                nc.sync.dma_start(
                    dp1t_sb[:, :, mt * P:(mt + 1) * P],
                    dp1t_d[:, :, mt * P:(mt + 1) * P],
                )
            # x split across both DMA queues, per chunk, so the first
            # matmuls wait on ~1MB instead of the whole 4MB
            for t2 in range(NTCH):
                tsl = slice(t2 * TCH, (t2 + 1) * TCH)
                nc.gpsimd.dma_start(
                    xt_sb[:, :KD // 2, tsl], xt_d[:, :KD // 2, tsl]
                )
                nc.sync.dma_start(
                    xt_sb[:, KD // 2:, tsl], xt_d[:, KD // 2:, tsl]
                )

            for t2 in range(NTCH):
                tsl = slice(t2 * TCH, (t2 + 1) * TCH)
                # gate logits, transposed: psum[E, cw] = gw.T @ x
                psl = pslp.tile([E, TCH], f32, tag="psl")
                for ko in range(KD):
                    nc.tensor.matmul(
                        psl,
                        gwt_sb[:, ko, :],
                        xt_sb[:, ko, tsl],
                        start=(ko == 0),
                        stop=(ko == KD - 1),
                    )
                nc.scalar.activation(lt_sb[:, tsl], psl, AF.Identity)
                # difficulty hidden: h1T[mt] = W1[:,mt].T @ x, relu
                h1T = h1p.tile([P, MH1, TCH], f32r, tag="h1T")
                for mt in range(MH1):
                    ps = psb.tile([P, TCH], f32, tag="psb")
                    for ko in range(KD):
                        nc.tensor.matmul(
                            ps,
                            dp1t_sb[:, ko, mt * P:(mt + 1) * P],
                            xt_sb[:, ko, tsl],
                            start=(ko == 0),
                            stop=(ko == KD - 1),
                        )
                    nc.scalar.activation(
                        h1T[:, mt, :], ps, AF.Relu, bias=dpb1_sb[:, mt:mt + 1]
                    )
                # z (pre-softplus difficulty logit), transposed: [1, cw]
                pse = psep.tile([1, TCH], f32, tag="pse")
                for ko in range(MH1):
                    nc.tensor.matmul(
                        pse,
                        dp2t_sb[:, ko:ko + 1],
                        h1T[:, ko, :],
                        start=(ko == 0),
                        stop=(ko == MH1 - 1),
                    )
                nc.scalar.activation(z_sb[:, tsl], pse, AF.Identity)

            nc.sync.dma_start(z_d[:], z_sb[:])
            nc.sync.dma_start(lt_d[:], lt_sb[:])

    nc.compile()
    names = {k: _ap_name(v) for k, v in {
        "xt": xt_d, "dp1t": dp1t_d, "dp2t": dp2t_d, "gwt": gwt_d,
        "dpb1": dpb1_d, "zt": z_d, "lt": lt_d,
    }.items()}
    return nc, names


def build_ffn_nc(C):
    """Phase-2 module: one expert FFN over C gathered tokens per core.

    All matmuls use up-to-512-wide moving operands (tokens): mm1 computes
    hid^T = W1^T x^T, mm2 computes y^T = W2^T hid^T (weights stationary).
    """
    import concourse.mybir as mybir
    import concourse.tile as tile
    from concourse import bacc
    from contextlib import ExitStack

    f32 = mybir.dt.float32
    f32r = mybir.dt.float32r
    AF = mybir.ActivationFunctionType

    chunks = []
    c0 = 0
    while c0 < C:
        cw = min(TCH, C - c0)
        chunks.append((c0, cw))
        c0 += cw

    nc = bacc.Bacc(None, target_bir_lowering=False, debug=False)

    with tile.TileContext(nc) as tc:
        with ExitStack() as ctx:
            dram = ctx.enter_context(tc.tile_pool(name="dram", bufs=1, space="DRAM"))
            xg_d = dram.tile([P, KD, C], f32r, kind="ExternalInput", name="xg")
            w1_d = dram.tile([P, KH, KD, P], f32r, kind="ExternalInput", name="w1g")
            w2_d = dram.tile([ND, P, KH, P], f32r, kind="ExternalInput", name="w2g")
            eb1_d = dram.tile([P, KH], f32, kind="ExternalInput", name="eb1g")
            yt_d = dram.tile([P, ND, C], f32, kind="ExternalOutput", name="yt")

            const = ctx.enter_context(tc.tile_pool(name="const", bufs=1))
            xgp = ctx.enter_context(tc.tile_pool(name="xgp", bufs=2))
            hidp = ctx.enter_context(tc.tile_pool(name="hidp", bufs=1))
            w2p = ctx.enter_context(tc.tile_pool(name="w2p", bufs=3))
            ytp = ctx.enter_context(tc.tile_pool(name="ytp", bufs=2))
            psb = ctx.enter_context(tc.tile_pool(name="psb", bufs=3, space="PSUM"))
            ps2p = ctx.enter_context(tc.tile_pool(name="ps2p", bufs=3, space="PSUM"))

            w1_sb = const.tile([P, KH, KD, P], f32r, tag="w1")
            eb1_sb = const.tile([P, KH], f32, tag="eb1")
            nc.sync.dma_start(eb1_sb[:], eb1_d[:])
            # W1 resident (64KB/partition), loaded per-ht so matmuls can
            # start before the whole 8MB lands
            for ht in range(KH):
                eng = nc.sync if ht % 2 == 0 else nc.gpsimd
                eng.dma_start(w1_sb[:, ht], w1_d[:, ht])

            for (c0, cw) in chunks:
                csl = slice(c0, c0 + cw)
                xg_sb = xgp.tile([P, KD, TCH], f32r, tag="xg")
                nc.gpsimd.dma_start(xg_sb[:, :, :cw], xg_d[:, :, csl])
                hidT = hidp.tile([P, KH, TCH], f32r, tag="hid")
                for ht in range(KH):
                    ps = psb.tile([P, TCH], f32, tag="psb")
                    for ko in range(KD):
                        nc.tensor.matmul(
                            ps[:, :cw],
                            w1_sb[:, ht, ko, :],
                            xg_sb[:, ko, :cw],
                            start=(ko == 0),
                            stop=(ko == KD - 1),
                        )
                    nc.scalar.activation(
                        hidT[:, ht, :cw], ps[:, :cw], AF.Relu,
                        bias=eb1_sb[:, ht:ht + 1],
                    )
                yt_sb = ytp.tile([P, ND, TCH], f32, tag="yt")
                for dt in range(ND):
                    w2s = w2p.tile([P, KH, P], f32r, tag="w2")
                    nc.sync.dma_start(w2s[:], w2_d[dt])
                    ps2 = ps2p.tile([P, TCH], f32, tag="ps2")
                    for ko in range(KH):
                        nc.tensor.matmul(
                            ps2[:, :cw],
                            w2s[:, ko, :],
                            hidT[:, ko, :cw],
                            start=(ko == 0),
                            stop=(ko == KH - 1),
                        )
                    nc.vector.tensor_copy(yt_sb[:, dt, :cw], ps2[:, :cw])
                    nc.gpsimd.dma_start(yt_d[:, dt, csl], yt_sb[:, dt, :cw])

    nc.compile()
    names = {k: _ap_name(v) for k, v in {
        "xg": xg_d, "w1g": w1_d, "w2g": w2_d, "eb1g": eb1_d, "yt": yt_d,
    }.items()}
    return nc, names


def prep_routing_in_maps(x, dp_w1, dp_b1, dp_w2, gate_w):
    f32 = np.float32
    xf = np.ascontiguousarray(x.reshape(T_FULL, D).astype(f32, copy=False))
    dp1t = np.ascontiguousarray(dp_w1.reshape(KD, P, H1).transpose(1, 0, 2))
    dp2t = np.ascontiguousarray(dp_w2[:, 0].reshape(MH1, P).T)
    gwt = np.ascontiguousarray(gate_w.reshape(KD, P, E).transpose(1, 0, 2))
    dpb1 = np.ascontiguousarray(dp_b1.reshape(MH1, P).T)
    shared = {"dp1t": dp1t, "dp2t": dp2t, "gwt": gwt, "dpb1": dpb1}
    in_maps = []
    for c in range(N_CORES):
        xc = xf[c * TC:(c + 1) * TC]
        xt = np.ascontiguousarray(xc.T.reshape(KD, P, TC).transpose(1, 0, 2))
        in_maps.append({"xt": xt, **shared})
    return in_maps, xf


def finish_routing(z, logits, xf, gate_w, gate_b, dp_w1, dp_b1, dp_w2, dp_b2):
    """Host fp64 finisher for the discrete routing decisions.

    z, logits carry fp32r matmul noise (~3e-4 worst case). Any token whose
    decision sits within RISK_MARGIN of a boundary is recomputed exactly in
    fp64, so k / top-k selections match the fp32 reference.
    """
    T = len(z)
    z = z.astype(np.float64) + float(dp_b2[0])
    logits = logits.astype(np.float64) + gate_b.astype(np.float64)[None, :]
    xf64 = None

    # exact recompute helpers
    def exact_z(tok):
        h = np.maximum(xf64[tok] @ dp_w1.astype(np.float64)
                       + dp_b1.astype(np.float64), 0)
        return h @ dp_w2.astype(np.float64)[:, 0] + float(dp_b2[0])

    def exact_logits(tok):
        return xf64[tok] @ gate_w.astype(np.float64) \
            + gate_b.astype(np.float64)[None, :]

    step = (TH_HI - TH_LO) / (2 * (E - MIN_E))
    zts = [np.log(np.expm1(TH_LO + (2 * i - 1) * step)) for i in (1, 2, 3)]

    risk_z = np.zeros(T, bool)
    for zt in zts:
        risk_z |= np.abs(z - zt) < RISK_MARGIN
    if risk_z.any():
        xf64 = xf.astype(np.float64)
        tok = np.nonzero(risk_z)[0]
        z[tok] = exact_z(tok)

    sl = np.sort(logits, axis=1)
    gaps = np.diff(sl, axis=1)
    risk_l = (gaps < RISK_MARGIN).any(axis=1)
    if risk_l.any():
        if xf64 is None:
            xf64 = xf.astype(np.float64)
        tok = np.nonzero(risk_l)[0]
        logits[tok] = exact_logits(tok)

    # k = round(1 + 3*clip((softplus(z)-0.5)/1.5, 0, 1)) via monotone
    # z-thresholds; middle threshold strict (RNE rounds kraw=2.5 down to 2)
    k = 1 + (z >= zts[0]).astype(np.int64) + (z > zts[1]) + (z >= zts[2])

    order = np.argsort(-logits, axis=1, kind="stable")
    rank = np.empty_like(order)
    rank[np.arange(T)[:, None], order] = np.arange(E)[None, :]
    sel = rank < k[:, None]

    m = logits.max(axis=1, keepdims=True)
    ex = np.where(sel, np.exp(logits - m), 0.0)
    dense_w = ex / ex.sum(axis=1, keepdims=True)
    return dense_w


def dispatch(dense_w):
    """Group tokens by assigned expert, split each expert across 2 cores."""
    halves = []
    for e in range(E):
        idx = np.nonzero(dense_w[:, e] > 0)[0]
        h = (len(idx) + 1) // 2
        halves.append(idx[:h])
        halves.append(idx[h:])
    cmax = max((len(h) for h in halves), default=1)
    C = max(((cmax + P - 1) // P) * P, P)
    return halves, C


def prep_ffn_in_maps(xf, halves, C, ew1, eb1, ew2):
    w1ts = [np.ascontiguousarray(
        ew1[e].reshape(KD, P, KH, P).transpose(1, 2, 0, 3)) for e in range(E)]
    w2ts = [np.ascontiguousarray(
        ew2[e].reshape(KH, P, ND, P).transpose(2, 1, 0, 3)) for e in range(E)]
    eb1ts = [np.ascontiguousarray(eb1[e].reshape(KH, P).T) for e in range(E)]
    in_maps = []
    for c in range(N_CORES):
        e = c // 2
        tok = halves[c]
        xp = np.zeros((C, D), dtype=np.float32)
        xp[:len(tok)] = xf[tok]
        xg = np.ascontiguousarray(xp.T.reshape(KD, P, C).transpose(1, 0, 2))
        in_maps.append({
            "xg": xg, "w1g": w1ts[e], "w2g": w2ts[e], "eb1g": eb1ts[e],
        })
    return in_maps


def remap_names(in_maps, names):
    return [{names[k]: v for k, v in m.items()} for m in in_maps]


_BUILT = {}


def _get(key, builder):
    if key not in _BUILT:
        _BUILT[key] = builder()
    return _BUILT[key]


def kernel(x, gate_w, gate_b, dp_w1, dp_b1, dp_w2, dp_b2, ew1, eb1, ew2, eb2,
           trace=False):
    from concourse.bass_utils import run_bass_kernel_spmd

    cores = list(range(N_CORES))

    # ---- phase 1: routing nets on device ----
    nc1, names1 = _get("routing", build_routing_nc)
    in1, xf = prep_routing_in_maps(x, dp_w1, dp_b1, dp_w2, gate_w)
    res1 = run_bass_kernel_spmd(nc1, remap_names(in1, names1), cores,
                                trace=trace)
    z = np.concatenate([r[names1["zt"]][0] for r in res1.results])
    logits = np.concatenate([r[names1["lt"]].T for r in res1.results])

    # ---- host: finish routing (fp64 + exact boundary fixups) ----
    dense_w = finish_routing(
        z, logits, xf, gate_w, gate_b, dp_w1, dp_b1, dp_w2, dp_b2
    )
    halves, C = dispatch(dense_w)

    # ---- phase 2: expert FFNs ----
    nc2, names2 = _get(("ffn", C), lambda: build_ffn_nc(C))
    in2 = prep_ffn_in_maps(xf, halves, C, ew1, eb1, ew2)
    res2 = run_bass_kernel_spmd(nc2, remap_names(in2, names2), cores,
                                trace=trace)

    # ---- host combine (scatter-add with routing weights) ----
    out = np.zeros((T_FULL, D), dtype=np.float64)
    for c in range(N_CORES):
        e = c // 2
        tok = halves[c]
        if len(tok) == 0:
            continue
        yt = res2.results[c][names2["yt"]]            # [P, ND, C]
        yg = yt.transpose(2, 1, 0).reshape(C, D)[:len(tok)]
        out[tok] += dense_w[tok, e, None] * yg
    if np.any(eb2):
        out += dense_w @ eb2.astype(np.float64)
    out = out.astype(np.float32).reshape(4, 2048, D)
    if trace:
        return out, (res1, res2)
    return out


# revision 15
# speedup vs baseline: 1.0783x; 1.0783x over previous
"""Trainium2 Bass kernel for nn_ConditionalMoELayer (expert-parallel, sparse).

Two-phase design following the expert-parallel sharding hint:

Phase 1 (routing nets, data-parallel): the 8192 tokens are split across the
8 cores (1024 each). Each core runs the difficulty net and the gate on its
tokens in fp32r and ships the raw difficulty logit z and gate logits back.
The host finishes the (tiny, [8192 x 4]) discrete routing math in fp64:
k(z) via monotone softplus-inverted thresholds, top-k by rank, masked
softmax -> dense_w. Tokens whose z or logit-gap sits within 1e-3 of a
decision boundary (a handful) are recomputed exactly in fp64 so the
discrete decisions match the fp32 reference despite fp32r matmul noise.

Host dispatch (the "all-to-all"): tokens are grouped by assigned expert
(dense_w[t,e] > 0); each expert's token set is split over 2 cores
(4 experts x 2 = 8 cores), zero-padded to a common static capacity C.

Phase 2 (expert FFN, expert-parallel): each core holds ONE expert's weights
and computes y = relu(x @ W1 + b1) @ W2 for its gathered tokens in fp32r
(single-pass PE, full rate). All matmuls have 512-wide moving operands so
the 4-byte weight loads stay hidden; activations ride the gpsimd DMA queue
so they never wait behind weight loads. Host scales rows by dense_w and
scatter-adds into the output (token sets within one expert are disjoint).

Only ~40% of token-expert pairs are active, so phase 2 does ~2.5x fewer
FLOPs than the dense equivalent.
"""

import numpy as np

P = 128          # partitions
D = 1024         # d_model
H = 2048         # expert hidden
E = 4            # experts
H1 = 512         # difficulty-net hidden
T_FULL = 8192    # total tokens
N_CORES = 8
TC = T_FULL // N_CORES   # tokens per core in phase 1

KD = D // P      # 8   k-subtiles over D
KH = H // P      # 16  k-subtiles over H
MH1 = H1 // P    # 4   m-tiles of difficulty hidden
NT = TC // P     # 8   token tiles per core (phase 1)
ND = D // P      # 8   d-tiles (phase 2 output)
TCH = 512        # token chunk for matmul moving operand
NTCH = TC // TCH # 2

TH_LO = 0.5
TH_HI = 2.0
MIN_E = 1
RISK_MARGIN = 1e-3   # fp32r logit noise is ~3e-4 worst case; 3x cushion


def _ap_name(t):
    return t.tensor.name if hasattr(t, "tensor") else t.name


def build_routing_nc():
    """Phase-1 module: difficulty-net z and gate logits (transposed out)."""
    import concourse.mybir as mybir
    import concourse.tile as tile
    from concourse import bacc
    from contextlib import ExitStack

    f32 = mybir.dt.float32
    f32r = mybir.dt.float32r
    AF = mybir.ActivationFunctionType

    nc = bacc.Bacc(None, target_bir_lowering=False, debug=False)

    with tile.TileContext(nc) as tc:
        with ExitStack() as ctx:
            dram = ctx.enter_context(tc.tile_pool(name="dram", bufs=1, space="DRAM"))
            xt_d = dram.tile([P, KD, TC], f32r, kind="ExternalInput", name="xt")
            dp1t_d = dram.tile([P, KD, H1], f32r, kind="ExternalInput", name="dp1t")
            dp2t_d = dram.tile([P, MH1], f32r, kind="ExternalInput", name="dp2t")
            gwt_d = dram.tile([P, KD, E], f32r, kind="ExternalInput", name="gwt")
            dpb1_d = dram.tile([P, MH1], f32, kind="ExternalInput", name="dpb1")
            z_d = dram.tile([1, TC], f32, kind="ExternalOutput", name="zt")
            lt_d = dram.tile([E, TC], f32, kind="ExternalOutput", name="lt")

            const = ctx.enter_context(tc.tile_pool(name="const", bufs=1))
            xtp = ctx.enter_context(tc.tile_pool(name="xtp", bufs=1))
            h1p = ctx.enter_context(tc.tile_pool(name="h1p", bufs=2))
            outp = ctx.enter_context(tc.tile_pool(name="outp", bufs=1))
            psb = ctx.enter_context(tc.tile_pool(name="psb", bufs=4, space="PSUM"))
            pslp = ctx.enter_context(tc.tile_pool(name="pslp", bufs=2, space="PSUM"))
            psep = ctx.enter_context(tc.tile_pool(name="psep", bufs=2, space="PSUM"))

            xt_sb = xtp.tile([P, KD, TC], f32r, tag="xt")
            dp1t_sb = const.tile([P, KD, H1], f32r, tag="dp1t")
            dp2t_sb = const.tile([P, MH1], f32r, tag="dp2t")
            gwt_sb = const.tile([P, KD, E], f32r, tag="gwt")
            dpb1_sb = const.tile([P, MH1], f32, tag="dpb1")
            z_sb = outp.tile([1, TC], f32, tag="zsb")
            lt_sb = outp.tile([E, TC], f32, tag="ltsb")

            nc.sync.dma_start(gwt_sb[:], gwt_d[:])
            nc.sync.dma_start(dpb1_sb[:], dpb1_d[:])
            nc.sync.dma_start(dp2t_sb[:], dp2t_d[:])
            for mt in range(MH1):
                nc.sync.dma_start(
                    dp1t_sb[:, :, mt * P:(mt + 1) * P],
                    dp1t_d[:, :, mt * P:(mt + 1) * P],
                )
            # x split across both DMA queues, per chunk, so the first
            # matmuls wait on ~1MB instead of the whole 4MB
            for t2 in range(NTCH):
                tsl = slice(t2 * TCH, (t2 + 1) * TCH)
                nc.gpsimd.dma_start(
                    xt_sb[:, :KD // 2, tsl], xt_d[:, :KD // 2, tsl]
                )
                nc.sync.dma_start(
                    xt_sb[:, KD // 2:, tsl], xt_d[:, KD // 2:, tsl]
                )

            for t2 in range(NTCH):
                tsl = slice(t2 * TCH, (t2 + 1) * TCH)
                # gate logits, transposed: psum[E, cw] = gw.T @ x
                psl = pslp.tile([E, TCH], f32, tag="psl")
                for ko in range(KD):
                    nc.tensor.matmul(
                        psl,
                        gwt_sb[:, ko, :],
                        xt_sb[:, ko, tsl],
                        start=(ko == 0),
                        stop=(ko == KD - 1),
                    )
                nc.scalar.activation(lt_sb[:, tsl], psl, AF.Identity)
                # difficulty hidden: h1T[mt] = W1[:,mt].T @ x, relu
                h1T = h1p.tile([P, MH1, TCH], f32r, tag="h1T")
                for mt in range(MH1):
                    ps = psb.tile([P, TCH], f32, tag="psb")
                    for ko in range(KD):
                        nc.tensor.matmul(
                            ps,
                            dp1t_sb[:, ko, mt * P:(mt + 1) * P],
                            xt_sb[:, ko, tsl],
                            start=(ko == 0),
                            stop=(ko == KD - 1),
                        )
                    nc.scalar.activation(
                        h1T[:, mt, :], ps, AF.Relu, bias=dpb1_sb[:, mt:mt + 1]
                    )
                # z (pre-softplus difficulty logit), transposed: [1, cw]
                pse = psep.tile([1, TCH], f32, tag="pse")
                for ko in range(MH1):
                    nc.tensor.matmul(
                        pse,
                        dp2t_sb[:, ko:ko + 1],
                        h1T[:, ko, :],
                        start=(ko == 0),
                        stop=(ko == MH1 - 1),
                    )
                nc.scalar.activation(z_sb[:, tsl], pse, AF.Identity)

            nc.sync.dma_start(z_d[:], z_sb[:])
            nc.sync.dma_start(lt_d[:], lt_sb[:])

    nc.compile()
    names = {k: _ap_name(v) for k, v in {
        "xt": xt_d, "dp1t": dp1t_d, "dp2t": dp2t_d, "gwt": gwt_d,
        "dpb1": dpb1_d, "zt": z_d, "lt": lt_d,
    }.items()}
    return nc, names


def build_ffn_nc(C):
    """Phase-2 module: one expert FFN over C gathered tokens per core.

    All matmuls use up-to-512-wide moving operands (tokens): mm1 computes
    hid^T = W1^T x^T, mm2 computes y^T = W2^T hid^T (weights stationary).
    """
    import concourse.mybir as mybir
    import concourse.tile as tile
    from concourse import bacc
    from contextlib import ExitStack

    f32 = mybir.dt.float32
    f32r = mybir.dt.float32r
    AF = mybir.ActivationFunctionType

    chunks = []
    c0 = 0
    while c0 < C:
        cw = min(TCH, C - c0)
        chunks.append((c0, cw))
        c0 += cw

    nc = bacc.Bacc(None, target_bir_lowering=False, debug=False)

    with tile.TileContext(nc) as tc:
        with ExitStack() as ctx:
            dram = ctx.enter_context(tc.tile_pool(name="dram", bufs=1, space="DRAM"))
            xg_d = dram.tile([P, KD, C], f32r, kind="ExternalInput", name="xg")
            w1_d = dram.tile([P, KH, KD, P], f32r, kind="ExternalInput", name="w1g")
            w2_d = dram.tile([ND, P, KH, P], f32r, kind="ExternalInput", name="w2g")
            eb1_d = dram.tile([P, KH], f32, kind="ExternalInput", name="eb1g")
            yt_d = dram.tile([P, ND, C], f32, kind="ExternalOutput", name="yt")

            const = ctx.enter_context(tc.tile_pool(name="const", bufs=1))
            xgp = ctx.enter_context(tc.tile_pool(name="xgp", bufs=2))
            hidp = ctx.enter_context(tc.tile_pool(name="hidp", bufs=1))
            w2p = ctx.enter_context(tc.tile_pool(name="w2p", bufs=3))
            ytp = ctx.enter_context(tc.tile_pool(name="ytp", bufs=2))
            psb = ctx.enter_context(tc.tile_pool(name="psb", bufs=3, space="PSUM"))
            ps2p = ctx.enter_context(tc.tile_pool(name="ps2p", bufs=3, space="PSUM"))

            w1_sb = const.tile([P, KH, KD, P], f32r, tag="w1")
            eb1_sb = const.tile([P, KH], f32, tag="eb1")
            nc.sync.dma_start(eb1_sb[:], eb1_d[:])
            # W1 resident (64KB/partition), loaded per-ht so matmuls can
            # start before the whole 8MB lands
            for ht in range(KH):
                nc.sync.dma_start(w1_sb[:, ht], w1_d[:, ht])

            for (c0, cw) in chunks:
                csl = slice(c0, c0 + cw)
                xg_sb = xgp.tile([P, KD, TCH], f32r, tag="xg")
                nc.gpsimd.dma_start(xg_sb[:, :, :cw], xg_d[:, :, csl])
                hidT = hidp.tile([P, KH, TCH], f32r, tag="hid")
                for ht in range(KH):
                    ps = psb.tile([P, TCH], f32, tag="psb")
                    for ko in range(KD):
                        nc.tensor.matmul(
                            ps[:, :cw],
                            w1_sb[:, ht, ko, :],
                            xg_sb[:, ko, :cw],
                            start=(ko == 0),
                            stop=(ko == KD - 1),
                        )
                    nc.scalar.activation(
                        hidT[:, ht, :cw], ps[:, :cw], AF.Relu,
                        bias=eb1_sb[:, ht:ht + 1],
                    )
                yt_sb = ytp.tile([P, ND, TCH], f32, tag="yt")
                for dt in range(ND):
                    w2s = w2p.tile([P, KH, P], f32r, tag="w2")
                    nc.sync.dma_start(w2s[:], w2_d[dt])
                    ps2 = ps2p.tile([P, TCH], f32, tag="ps2")
                    for ko in range(KH):
                        nc.tensor.matmul(
                            ps2[:, :cw],
                            w2s[:, ko, :],
                            hidT[:, ko, :cw],
                            start=(ko == 0),
                            stop=(ko == KH - 1),
                        )
                    nc.vector.tensor_copy(yt_sb[:, dt, :cw], ps2[:, :cw])
                    if dt % 4 == 3:
                        nc.gpsimd.dma_start(
                            yt_d[:, dt - 3:dt + 1, csl],
                            yt_sb[:, dt - 3:dt + 1, :cw],
                        )

    nc.compile()
    names = {k: _ap_name(v) for k, v in {
        "xg": xg_d, "w1g": w1_d, "w2g": w2_d, "eb1g": eb1_d, "yt": yt_d,
    }.items()}
    return nc, names


def prep_routing_in_maps(x, dp_w1, dp_b1, dp_w2, gate_w):
    f32 = np.float32
    xf = np.ascontiguousarray(x.reshape(T_FULL, D).astype(f32, copy=False))
    dp1t = np.ascontiguousarray(dp_w1.reshape(KD, P, H1).transpose(1, 0, 2))
    dp2t = np.ascontiguousarray(dp_w2[:, 0].reshape(MH1, P).T)
    gwt = np.ascontiguousarray(gate_w.reshape(KD, P, E).transpose(1, 0, 2))
    dpb1 = np.ascontiguousarray(dp_b1.reshape(MH1, P).T)
    shared = {"dp1t": dp1t, "dp2t": dp2t, "gwt": gwt, "dpb1": dpb1}
    in_maps = []
    for c in range(N_CORES):
        xc = xf[c * TC:(c + 1) * TC]
        xt = np.ascontiguousarray(xc.T.reshape(KD, P, TC).transpose(1, 0, 2))
        in_maps.append({"xt": xt, **shared})
    return in_maps, xf


def finish_routing(z, logits, xf, gate_w, gate_b, dp_w1, dp_b1, dp_w2, dp_b2):
    """Host fp64 finisher for the discrete routing decisions.

    z, logits carry fp32r matmul noise (~3e-4 worst case). Any token whose
    decision sits within RISK_MARGIN of a boundary is recomputed exactly in
    fp64, so k / top-k selections match the fp32 reference.
    """
    T = len(z)
    z = z.astype(np.float64) + float(dp_b2[0])
    logits = logits.astype(np.float64) + gate_b.astype(np.float64)[None, :]
    xf64 = None

    # exact recompute helpers
    def exact_z(tok):
        h = np.maximum(xf64[tok] @ dp_w1.astype(np.float64)
                       + dp_b1.astype(np.float64), 0)
        return h @ dp_w2.astype(np.float64)[:, 0] + float(dp_b2[0])

    def exact_logits(tok):
        return xf64[tok] @ gate_w.astype(np.float64) \
            + gate_b.astype(np.float64)[None, :]

    step = (TH_HI - TH_LO) / (2 * (E - MIN_E))
    zts = [np.log(np.expm1(TH_LO + (2 * i - 1) * step)) for i in (1, 2, 3)]

    risk_z = np.zeros(T, bool)
    for zt in zts:
        risk_z |= np.abs(z - zt) < RISK_MARGIN
    if risk_z.any():
        xf64 = xf.astype(np.float64)
        tok = np.nonzero(risk_z)[0]
        z[tok] = exact_z(tok)

    sl = np.sort(logits, axis=1)
    gaps = np.diff(sl, axis=1)
    risk_l = (gaps < RISK_MARGIN).any(axis=1)
    if risk_l.any():
        if xf64 is None:
            xf64 = xf.astype(np.float64)
        tok = np.nonzero(risk_l)[0]
        logits[tok] = exact_logits(tok)

    # k = round(1 + 3*clip((softplus(z)-0.5)/1.5, 0, 1)) via monotone
    # z-thresholds; middle threshold strict (RNE rounds kraw=2.5 down to 2)
    k = 1 + (z >= zts[0]).astype(np.int64) + (z > zts[1]) + (z >= zts[2])

    order = np.argsort(-logits, axis=1, kind="stable")
    rank = np.empty_like(order)
    rank[np.arange(T)[:, None], order] = np.arange(E)[None, :]
    sel = rank < k[:, None]

    m = logits.max(axis=1, keepdims=True)
    ex = np.where(sel, np.exp(logits - m), 0.0)
    dense_w = ex / ex.sum(axis=1, keepdims=True)
    return dense_w


def dispatch(dense_w):
    """Group tokens by assigned expert, split each expert across 2 cores."""
    halves = []
    for e in range(E):
        idx = np.nonzero(dense_w[:, e] > 0)[0]
        h = (len(idx) + 1) // 2
        halves.append(idx[:h])
        halves.append(idx[h:])
    cmax = max((len(h) for h in halves), default=1)
    C = max(((cmax + P - 1) // P) * P, P)
    return halves, C


def prep_ffn_in_maps(xf, halves, C, ew1, eb1, ew2):
    w1ts = [np.ascontiguousarray(
        ew1[e].reshape(KD, P, KH, P).transpose(1, 2, 0, 3)) for e in range(E)]
    w2ts = [np.ascontiguousarray(
        ew2[e].reshape(KH, P, ND, P).transpose(2, 1, 0, 3)) for e in range(E)]
    eb1ts = [np.ascontiguousarray(eb1[e].reshape(KH, P).T) for e in range(E)]
    in_maps = []
    for c in range(N_CORES):
        e = c // 2
        tok = halves[c]
        xp = np.zeros((C, D), dtype=np.float32)
        xp[:len(tok)] = xf[tok]
        xg = np.ascontiguousarray(xp.T.reshape(KD, P, C).transpose(1, 0, 2))
        in_maps.append({
            "xg": xg, "w1g": w1ts[e], "w2g": w2ts[e], "eb1g": eb1ts[e],
        })
    return in_maps


def remap_names(in_maps, names):
    return [{names[k]: v for k, v in m.items()} for m in in_maps]


_BUILT = {}


def _get(key, builder):
    if key not in _BUILT:
        _BUILT[key] = builder()
    return _BUILT[key]


def kernel(x, gate_w, gate_b, dp_w1, dp_b1, dp_w2, dp_b2, ew1, eb1, ew2, eb2,
           trace=False):
    from concourse.bass_utils import run_bass_kernel_spmd

    cores = list(range(N_CORES))

    # ---- phase 1: routing nets on device ----
    nc1, names1 = _get("routing", build_routing_nc)
    in1, xf = prep_routing_in_maps(x, dp_w1, dp_b1, dp_w2, gate_w)
    res1 = run_bass_kernel_spmd(nc1, remap_names(in1, names1), cores,
                                trace=trace)
    z = np.concatenate([r[names1["zt"]][0] for r in res1.results])
    logits = np.concatenate([r[names1["lt"]].T for r in res1.results])

    # ---- host: finish routing (fp64 + exact boundary fixups) ----
    dense_w = finish_routing(
        z, logits, xf, gate_w, gate_b, dp_w1, dp_b1, dp_w2, dp_b2
    )
    halves, C = dispatch(dense_w)

    # ---- phase 2: expert FFNs ----
    nc2, names2 = _get(("ffn", C), lambda: build_ffn_nc(C))
    in2 = prep_ffn_in_maps(xf, halves, C, ew1, eb1, ew2)
    res2 = run_bass_kernel_spmd(nc2, remap_names(in2, names2), cores,
                                trace=trace)

    # ---- host combine (scatter-add with routing weights) ----
    out = np.zeros((T_FULL, D), dtype=np.float64)
    for c in range(N_CORES):
        e = c // 2
        tok = halves[c]
        if len(tok) == 0:
            continue
        yt = res2.results[c][names2["yt"]]            # [P, ND, C]
        yg = yt.transpose(2, 1, 0).reshape(C, D)[:len(tok)]
        out[tok] += dense_w[tok, e, None] * yg
    if np.any(eb2):
        out += dense_w @ eb2.astype(np.float64)
    out = out.astype(np.float32).reshape(4, 2048, D)
    if trace:
        return out, (res1, res2)
    return out


# revision 16
# speedup vs baseline: 1.0897x; 1.0106x over previous
"""Trainium2 Bass kernel for nn_ConditionalMoELayer (expert-parallel, sparse).

Two-phase design following the expert-parallel sharding hint:

Phase 1 (routing nets, data-parallel): the 8192 tokens are split across the
8 cores (1024 each). Each core runs the difficulty net and the gate on its
tokens in fp32r and ships the raw difficulty logit z and gate logits back.
The host finishes the (tiny, [8192 x 4]) discrete routing math in fp64:
k(z) via monotone softplus-inverted thresholds, top-k by rank, masked
softmax -> dense_w. Tokens whose z or logit-gap sits within 1e-3 of a
decision boundary (a handful) are recomputed exactly in fp64 so the
discrete decisions match the fp32 reference despite fp32r matmul noise.

Host dispatch (the "all-to-all"): tokens are grouped by assigned expert
(dense_w[t,e] > 0); each expert's token set is split over 2 cores
(4 experts x 2 = 8 cores), zero-padded to a common static capacity C.

Phase 2 (expert FFN, expert-parallel): each core holds ONE expert's weights
and computes y = relu(x @ W1 + b1) @ W2 for its gathered tokens in fp32r
(single-pass PE, full rate). All matmuls have 512-wide moving operands so
the 4-byte weight loads stay hidden; activations ride the gpsimd DMA queue
so they never wait behind weight loads. Host scales rows by dense_w and
scatter-adds into the output (token sets within one expert are disjoint).

Only ~40% of token-expert pairs are active, so phase 2 does ~2.5x fewer
FLOPs than the dense equivalent.
"""

import numpy as np

P = 128          # partitions
D = 1024         # d_model
H = 2048         # expert hidden
E = 4            # experts
H1 = 512         # difficulty-net hidden
T_FULL = 8192    # total tokens
N_CORES = 8
TC = T_FULL // N_CORES   # tokens per core in phase 1

KD = D // P      # 8   k-subtiles over D
KH = H // P      # 16  k-subtiles over H
MH1 = H1 // P    # 4   m-tiles of difficulty hidden
NT = TC // P     # 8   token tiles per core (phase 1)
ND = D // P      # 8   d-tiles (phase 2 output)
TCH = 512        # token chunk for matmul moving operand
NTCH = TC // TCH # 2

TH_LO = 0.5
TH_HI = 2.0
MIN_E = 1
RISK_MARGIN = 1e-3   # fp32r logit noise is ~3e-4 worst case; 3x cushion


def _ap_name(t):
    return t.tensor.name if hasattr(t, "tensor") else t.name


def build_routing_nc():
    """Phase-1 module: difficulty-net z and gate logits (transposed out)."""
    import concourse.mybir as mybir
    import concourse.tile as tile
    from concourse import bacc
    from contextlib import ExitStack

    f32 = mybir.dt.float32
    f32r = mybir.dt.float32r
    AF = mybir.ActivationFunctionType

    nc = bacc.Bacc(None, target_bir_lowering=False, debug=False)

    with tile.TileContext(nc) as tc:
        with ExitStack() as ctx:
            dram = ctx.enter_context(tc.tile_pool(name="dram", bufs=1, space="DRAM"))
            xt_d = dram.tile([P, KD, TC], f32r, kind="ExternalInput", name="xt")
            dp1t_d = dram.tile([P, KD, H1], f32r, kind="ExternalInput", name="dp1t")
            dp2t_d = dram.tile([P, MH1], f32r, kind="ExternalInput", name="dp2t")
            gwt_d = dram.tile([P, KD, E], f32r, kind="ExternalInput", name="gwt")
            dpb1_d = dram.tile([P, MH1], f32, kind="ExternalInput", name="dpb1")
            z_d = dram.tile([1, TC], f32, kind="ExternalOutput", name="zt")
            lt_d = dram.tile([E, TC], f32, kind="ExternalOutput", name="lt")

            const = ctx.enter_context(tc.tile_pool(name="const", bufs=1))
            xtp = ctx.enter_context(tc.tile_pool(name="xtp", bufs=1))
            h1p = ctx.enter_context(tc.tile_pool(name="h1p", bufs=2))
            outp = ctx.enter_context(tc.tile_pool(name="outp", bufs=1))
            psb = ctx.enter_context(tc.tile_pool(name="psb", bufs=4, space="PSUM"))
            pslp = ctx.enter_context(tc.tile_pool(name="pslp", bufs=2, space="PSUM"))
            psep = ctx.enter_context(tc.tile_pool(name="psep", bufs=2, space="PSUM"))

            xt_sb = xtp.tile([P, KD, TC], f32r, tag="xt")
            dp1t_sb = const.tile([P, KD, H1], f32r, tag="dp1t")
            dp2t_sb = const.tile([P, MH1], f32r, tag="dp2t")
            gwt_sb = const.tile([P, KD, E], f32r, tag="gwt")
            dpb1_sb = const.tile([P, MH1], f32, tag="dpb1")
            z_sb = outp.tile([1, TC], f32, tag="zsb")
            lt_sb = outp.tile([E, TC], f32, tag="ltsb")

            nc.sync.dma_start(gwt_sb[:], gwt_d[:])
            nc.sync.dma_start(dpb1_sb[:], dpb1_d[:])
            # x split across both DMA queues, per chunk, ahead of the dp
            # weights, so the gate matmuls start ~5us in
            for t2 in range(NTCH):
                tsl = slice(t2 * TCH, (t2 + 1) * TCH)
                nc.gpsimd.dma_start(
                    xt_sb[:, :KD // 2, tsl], xt_d[:, :KD // 2, tsl]
                )
                nc.sync.dma_start(
                    xt_sb[:, KD // 2:, tsl], xt_d[:, KD // 2:, tsl]
                )
            nc.sync.dma_start(dp2t_sb[:], dp2t_d[:])
            for mt in range(MH1):
                nc.sync.dma_start(
                    dp1t_sb[:, :, mt * P:(mt + 1) * P],
                    dp1t_d[:, :, mt * P:(mt + 1) * P],
                )

            for t2 in range(NTCH):
                tsl = slice(t2 * TCH, (t2 + 1) * TCH)
                # gate logits, transposed: psum[E, cw] = gw.T @ x
                psl = pslp.tile([E, TCH], f32, tag="psl")
                for ko in range(KD):
                    nc.tensor.matmul(
                        psl,
                        gwt_sb[:, ko, :],
                        xt_sb[:, ko, tsl],
                        start=(ko == 0),
                        stop=(ko == KD - 1),
                    )
                nc.scalar.activation(lt_sb[:, tsl], psl, AF.Identity)
                # difficulty hidden: h1T[mt] = W1[:,mt].T @ x, relu
                h1T = h1p.tile([P, MH1, TCH], f32r, tag="h1T")
                for mt in range(MH1):
                    ps = psb.tile([P, TCH], f32, tag="psb")
                    for ko in range(KD):
                        nc.tensor.matmul(
                            ps,
                            dp1t_sb[:, ko, mt * P:(mt + 1) * P],
                            xt_sb[:, ko, tsl],
                            start=(ko == 0),
                            stop=(ko == KD - 1),
                        )
                    nc.scalar.activation(
                        h1T[:, mt, :], ps, AF.Relu, bias=dpb1_sb[:, mt:mt + 1]
                    )
                # z (pre-softplus difficulty logit), transposed: [1, cw]
                pse = psep.tile([1, TCH], f32, tag="pse")
                for ko in range(MH1):
                    nc.tensor.matmul(
                        pse,
                        dp2t_sb[:, ko:ko + 1],
                        h1T[:, ko, :],
                        start=(ko == 0),
                        stop=(ko == MH1 - 1),
                    )
                nc.scalar.activation(z_sb[:, tsl], pse, AF.Identity)

            nc.sync.dma_start(z_d[:], z_sb[:])
            nc.sync.dma_start(lt_d[:], lt_sb[:])

    nc.compile()
    names = {k: _ap_name(v) for k, v in {
        "xt": xt_d, "dp1t": dp1t_d, "dp2t": dp2t_d, "gwt": gwt_d,
        "dpb1": dpb1_d, "zt": z_d, "lt": lt_d,
    }.items()}
    return nc, names


def build_ffn_nc(C):
    """Phase-2 module: one expert FFN over C gathered tokens per core.

    All matmuls use up-to-512-wide moving operands (tokens): mm1 computes
    hid^T = W1^T x^T, mm2 computes y^T = W2^T hid^T (weights stationary).
    """
    import concourse.mybir as mybir
    import concourse.tile as tile
    from concourse import bacc
    from contextlib import ExitStack

    f32 = mybir.dt.float32
    f32r = mybir.dt.float32r
    AF = mybir.ActivationFunctionType

    chunks = []
    c0 = 0
    while c0 < C:
        cw = min(TCH, C - c0)
        chunks.append((c0, cw))
        c0 += cw

    nc = bacc.Bacc(None, target_bir_lowering=False, debug=False)

    with tile.TileContext(nc) as tc:
        with ExitStack() as ctx:
            dram = ctx.enter_context(tc.tile_pool(name="dram", bufs=1, space="DRAM"))
            xg_d = dram.tile([P, KD, C], f32r, kind="ExternalInput", name="xg")
            w1_d = dram.tile([P, KH, KD, P], f32r, kind="ExternalInput", name="w1g")
            w2_d = dram.tile([ND, P, KH, P], f32r, kind="ExternalInput", name="w2g")
            eb1_d = dram.tile([P, KH], f32, kind="ExternalInput", name="eb1g")
            yt_d = dram.tile([P, ND, C], f32, kind="ExternalOutput", name="yt")

            const = ctx.enter_context(tc.tile_pool(name="const", bufs=1))
            xgp = ctx.enter_context(tc.tile_pool(name="xgp", bufs=2))
            hidp = ctx.enter_context(tc.tile_pool(name="hidp", bufs=1))
            w2p = ctx.enter_context(tc.tile_pool(name="w2p", bufs=3))
            ytp = ctx.enter_context(tc.tile_pool(name="ytp", bufs=2))
            psb = ctx.enter_context(tc.tile_pool(name="psb", bufs=3, space="PSUM"))
            ps2p = ctx.enter_context(tc.tile_pool(name="ps2p", bufs=3, space="PSUM"))

            w1_sb = const.tile([P, KH, KD, P], f32r, tag="w1")
            eb1_sb = const.tile([P, KH], f32, tag="eb1")
            nc.sync.dma_start(eb1_sb[:], eb1_d[:])
            # chunk-0 activations jump the weight queue so the first matmul
            # can issue ~6us in; W1 streams behind per-ht
            xg_tiles = []
            for ci, (c0, cw) in enumerate(chunks):
                xg_sb = xgp.tile([P, KD, TCH], f32r, tag="xg", name=f"xg{ci}")
                xg_tiles.append(xg_sb)
            nc.sync.dma_start(xg_tiles[0][:, :, :chunks[0][1]],
                              xg_d[:, :, :chunks[0][1]])
            # W1 resident (64KB/partition), loaded per-ht so matmuls can
            # start before the whole 8MB lands
            for ht in range(KH):
                nc.sync.dma_start(w1_sb[:, ht], w1_d[:, ht])

            for ci, (c0, cw) in enumerate(chunks):
                csl = slice(c0, c0 + cw)
                xg_sb = xg_tiles[ci]
                if ci > 0:
                    nc.gpsimd.dma_start(xg_sb[:, :, :cw], xg_d[:, :, csl])
                hidT = hidp.tile([P, KH, TCH], f32r, tag="hid")
                for ht in range(KH):
                    ps = psb.tile([P, TCH], f32, tag="psb")
                    for ko in range(KD):
                        nc.tensor.matmul(
                            ps[:, :cw],
                            w1_sb[:, ht, ko, :],
                            xg_sb[:, ko, :cw],
                            start=(ko == 0),
                            stop=(ko == KD - 1),
                        )
                    nc.scalar.activation(
                        hidT[:, ht, :cw], ps[:, :cw], AF.Relu,
                        bias=eb1_sb[:, ht:ht + 1],
                    )
                yt_sb = ytp.tile([P, ND, TCH], f32, tag="yt")
                for dt in range(ND):
                    w2s = w2p.tile([P, KH, P], f32r, tag="w2")
                    nc.sync.dma_start(w2s[:], w2_d[dt])
                    ps2 = ps2p.tile([P, TCH], f32, tag="ps2")
                    for ko in range(KH):
                        nc.tensor.matmul(
                            ps2[:, :cw],
                            w2s[:, ko, :],
                            hidT[:, ko, :cw],
                            start=(ko == 0),
                            stop=(ko == KH - 1),
                        )
                    nc.vector.tensor_copy(yt_sb[:, dt, :cw], ps2[:, :cw])
                nc.gpsimd.dma_start(yt_d[:, :, csl], yt_sb[:, :, :cw])

    nc.compile()
    names = {k: _ap_name(v) for k, v in {
        "xg": xg_d, "w1g": w1_d, "w2g": w2_d, "eb1g": eb1_d, "yt": yt_d,
    }.items()}
    return nc, names


def prep_routing_in_maps(x, dp_w1, dp_b1, dp_w2, gate_w):
    f32 = np.float32
    xf = np.ascontiguousarray(x.reshape(T_FULL, D).astype(f32, copy=False))
    dp1t = np.ascontiguousarray(dp_w1.reshape(KD, P, H1).transpose(1, 0, 2))
    dp2t = np.ascontiguousarray(dp_w2[:, 0].reshape(MH1, P).T)
    gwt = np.ascontiguousarray(gate_w.reshape(KD, P, E).transpose(1, 0, 2))
    dpb1 = np.ascontiguousarray(dp_b1.reshape(MH1, P).T)
    shared = {"dp1t": dp1t, "dp2t": dp2t, "gwt": gwt, "dpb1": dpb1}
    in_maps = []
    for c in range(N_CORES):
        xc = xf[c * TC:(c + 1) * TC]
        xt = np.ascontiguousarray(xc.T.reshape(KD, P, TC).transpose(1, 0, 2))
        in_maps.append({"xt": xt, **shared})
    return in_maps, xf


def finish_routing(z, logits, xf, gate_w, gate_b, dp_w1, dp_b1, dp_w2, dp_b2):
    """Host fp64 finisher for the discrete routing decisions.

    z, logits carry fp32r matmul noise (~3e-4 worst case). Any token whose
    decision sits within RISK_MARGIN of a boundary is recomputed exactly in
    fp64, so k / top-k selections match the fp32 reference.
    """
    T = len(z)
    z = z.astype(np.float64) + float(dp_b2[0])
    logits = logits.astype(np.float64) + gate_b.astype(np.float64)[None, :]
    xf64 = None

    # exact recompute helpers
    def exact_z(tok):
        h = np.maximum(xf64[tok] @ dp_w1.astype(np.float64)
                       + dp_b1.astype(np.float64), 0)
        return h @ dp_w2.astype(np.float64)[:, 0] + float(dp_b2[0])

    def exact_logits(tok):
        return xf64[tok] @ gate_w.astype(np.float64) \
            + gate_b.astype(np.float64)[None, :]

    step = (TH_HI - TH_LO) / (2 * (E - MIN_E))
    zts = [np.log(np.expm1(TH_LO + (2 * i - 1) * step)) for i in (1, 2, 3)]

    risk_z = np.zeros(T, bool)
    for zt in zts:
        risk_z |= np.abs(z - zt) < RISK_MARGIN
    if risk_z.any():
        xf64 = xf.astype(np.float64)
        tok = np.nonzero(risk_z)[0]
        z[tok] = exact_z(tok)

    sl = np.sort(logits, axis=1)
    gaps = np.diff(sl, axis=1)
    risk_l = (gaps < RISK_MARGIN).any(axis=1)
    if risk_l.any():
        if xf64 is None:
            xf64 = xf.astype(np.float64)
        tok = np.nonzero(risk_l)[0]
        logits[tok] = exact_logits(tok)

    # k = round(1 + 3*clip((softplus(z)-0.5)/1.5, 0, 1)) via monotone
    # z-thresholds; middle threshold strict (RNE rounds kraw=2.5 down to 2)
    k = 1 + (z >= zts[0]).astype(np.int64) + (z > zts[1]) + (z >= zts[2])

    order = np.argsort(-logits, axis=1, kind="stable")
    rank = np.empty_like(order)
    rank[np.arange(T)[:, None], order] = np.arange(E)[None, :]
    sel = rank < k[:, None]

    m = logits.max(axis=1, keepdims=True)
    ex = np.where(sel, np.exp(logits - m), 0.0)
    dense_w = ex / ex.sum(axis=1, keepdims=True)
    return dense_w


def dispatch(dense_w):
    """Group tokens by assigned expert, split each expert across 2 cores."""
    halves = []
    for e in range(E):
        idx = np.nonzero(dense_w[:, e] > 0)[0]
        h = (len(idx) + 1) // 2
        halves.append(idx[:h])
        halves.append(idx[h:])
    cmax = max((len(h) for h in halves), default=1)
    C = max(((cmax + P - 1) // P) * P, P)
    return halves, C


def prep_ffn_in_maps(xf, halves, C, ew1, eb1, ew2):
    w1ts = [np.ascontiguousarray(
        ew1[e].reshape(KD, P, KH, P).transpose(1, 2, 0, 3)) for e in range(E)]
    w2ts = [np.ascontiguousarray(
        ew2[e].reshape(KH, P, ND, P).transpose(2, 1, 0, 3)) for e in range(E)]
    eb1ts = [np.ascontiguousarray(eb1[e].reshape(KH, P).T) for e in range(E)]
    in_maps = []
    for c in range(N_CORES):
        e = c // 2
        tok = halves[c]
        xp = np.zeros((C, D), dtype=np.float32)
        xp[:len(tok)] = xf[tok]
        xg = np.ascontiguousarray(xp.T.reshape(KD, P, C).transpose(1, 0, 2))
        in_maps.append({
            "xg": xg, "w1g": w1ts[e], "w2g": w2ts[e], "eb1g": eb1ts[e],
        })
    return in_maps


def remap_names(in_maps, names):
    return [{names[k]: v for k, v in m.items()} for m in in_maps]


_BUILT = {}


def _get(key, builder):
    if key not in _BUILT:
        _BUILT[key] = builder()
    return _BUILT[key]


def kernel(x, gate_w, gate_b, dp_w1, dp_b1, dp_w2, dp_b2, ew1, eb1, ew2, eb2,
           trace=False):
    from concourse.bass_utils import run_bass_kernel_spmd

    cores = list(range(N_CORES))

    # ---- phase 1: routing nets on device ----
    nc1, names1 = _get("routing", build_routing_nc)
    in1, xf = prep_routing_in_maps(x, dp_w1, dp_b1, dp_w2, gate_w)
    res1 = run_bass_kernel_spmd(nc1, remap_names(in1, names1), cores,
                                trace=trace)
    z = np.concatenate([r[names1["zt"]][0] for r in res1.results])
    logits = np.concatenate([r[names1["lt"]].T for r in res1.results])

    # ---- host: finish routing (fp64 + exact boundary fixups) ----
    dense_w = finish_routing(
        z, logits, xf, gate_w, gate_b, dp_w1, dp_b1, dp_w2, dp_b2
    )
    halves, C = dispatch(dense_w)

    # ---- phase 2: expert FFNs ----
    nc2, names2 = _get(("ffn", C), lambda: build_ffn_nc(C))
    in2 = prep_ffn_in_maps(xf, halves, C, ew1, eb1, ew2)
    res2 = run_bass_kernel_spmd(nc2, remap_names(in2, names2), cores,
                                trace=trace)

    # ---- host combine (scatter-add with routing weights) ----
    out = np.zeros((T_FULL, D), dtype=np.float64)
    for c in range(N_CORES):
        e = c // 2
        tok = halves[c]
        if len(tok) == 0:
            continue
        yt = res2.results[c][names2["yt"]]            # [P, ND, C]
        yg = yt.transpose(2, 1, 0).reshape(C, D)[:len(tok)]
        out[tok] += dense_w[tok, e, None] * yg
    if np.any(eb2):
        out += dense_w @ eb2.astype(np.float64)
    out = out.astype(np.float32).reshape(4, 2048, D)
    if trace:
        return out, (res1, res2)
    return out
